# revision 24
# baseline (speedup 1.0000x reference)
"""Trainium2 Bass kernel for nn_Decoder_1357209666110.

LSTM decoder with attention + pointer scatter over an extended vocab.
Sharding: data-parallel over batch (4 of 32 batches per core, 8 cores).
All per-step state, attention, and the scatter are batch-local, so the
kernel needs no collectives.

Per core:
  phase 0: memories = enc @ W_enc (+b) in both [s,h] and [h,s] layouts.
  recurrence (32 sequential steps): fused-gates matmul (fp16 operands,
    fp32 PSUM accumulate), LSTM via tanh-only activations, attention
    energy/softmax/context, transposed state kept as matmul stationaries.
  phase B: li = tanh([h|ctx] @ W_cat), big logit matmul against W_out
    tiles (fp16), where(x==0, NEG, x), dense writes in [b, v, t] layout,
    then the pointer-scatter merge via host-compacted unique positions
    and one indirect scatter-add DMA of per-position deltas.

Host side: weight folding (W_red/W_ih -> W_ex/W_cx), G1 precompute,
enc transposes, scatter-index compaction, and final layout assembly.
"""

import sys

if "/opt/trn_rl_repo" not in sys.path:
    sys.path.insert(0, "/opt/trn_rl_repo")

import numpy as np

V, E, H, B, T, S, NOOV = 45000, 300, 512, 32, 32, 400, 30
NEG = np.float32(-1e12)
VE = V + NOOV            # 45030 true extended vocab
VP = 45056               # 352 * 128 padded vocab
NVC = VP // 128          # 352 vocab chunks
SP = 512                 # padded source length
NB = 4                   # batches per core
NC = 8                   # cores
BT = NB * T              # 128 rows per core
U = 512                  # padded unique scatter positions per batch

_BUILT = None            # (nc, input_names) cache — compile once per process


def _f16(x):
    return np.ascontiguousarray(x, dtype=np.float16)


def _f32(x):
    return np.ascontiguousarray(x, dtype=np.float32)


def _tf32(x):
    u = np.ascontiguousarray(x, np.float32).view(np.uint32)
    r = ((u + 0x1000 + ((u >> 13) & 1)) & 0xFFFFE000).astype(np.uint32)
    return r.view(np.float32)


def _build():
    import concourse.bass as bass
    import concourse.tile as tile
    from concourse import bacc, mybir

    dt = mybir.dt
    AF = mybir.ActivationFunctionType
    OP = mybir.AluOpType

    nc = bacc.Bacc("TRN2", target_bir_lowering=False, debug=False,
                   num_devices=NC)

    def din(name, shape, d):
        return nc.dram_tensor(name, list(shape), d, kind="ExternalInput").ap()

    # --- inputs (per-core data via in_maps) ---
    enc_tT = din("enc_tT", [128, 4, NB, SP], dt.float16)     # encT tiles
    w_enc = din("w_enc", [128, 4, H], dt.float16)            # W_enc lhsT tiles
    benc_row = din("benc_row", [1, H], dt.float16)           # bias row (N-dir)
    benc_chk = din("benc_chk", [1, 4, 128], dt.float16)      # bias chunks (M-dir)
    ones_row = din("ones_row", [1, SP], dt.float16)
    w_hc = din("w_hc", [128, 8, 4 * H], dt.float16)          # [W_hh; W_cx] tiles
    g1 = din("g1", [128, 16, NB, T], dt.float16)             # G1^T tiles
    hist0 = din("hist0", [128, 8, NB], dt.float16)           # [h0T | 0] init
    c0_2 = din("c0_2", [128, 4, NB], dt.float32)             # 2*c0 transposed
    maskneg = din("maskneg", [NB, SP], dt.float32)           # 0 / -1e12 rows
    id4f = din("id4f", [1, 4, 4], dt.float16)                # eye(4) in free dims
    id1 = din("id1", [1, 1], dt.float32)                     # [1,1] identity
    id4f32 = din("id4f32", [4, 4], dt.float32)
    w_cat = din("w_cat", [128, 8, H], dt.float16)
    bcat_row = din("bcat_row", [1, H], dt.float16)
    w_out_t = din("w_out_t", [NVC, 128, 4, 128], dt.float16)  # streamed tiles
    wg = din("wg", [NB, 128, 4, U], dt.float16)              # W_out[:, U_b] tiles
    a_oh = din("a_oh", [NB, 128, 4, U], dt.float32r)         # one-hot S->U
    id4 = din("id4", [4, 4], dt.float16)
    id32f = din("id32f", [32, 32], dt.float32)
    id128 = din("id128", [128, 128], dt.float16)

    out_scr = nc.dram_tensor("out_scr", [NB, VP, T], dt.float32,
                             kind="ExternalOutput").ap()
    e2f_out = nc.dram_tensor("e2f_out", [128, NB, 4, T], dt.float32,
                             kind="ExternalOutput").ap()

    from contextlib import ExitStack
    with tile.TileContext(nc) as tc, ExitStack() as ctx:
        cp = ctx.enter_context(tc.tile_pool(name="consts", bufs=1))
        rp = ctx.enter_context(tc.tile_pool(name="resident", bufs=1))
        wk = ctx.enter_context(tc.tile_pool(name="work", bufs=2))
        pp = ctx.enter_context(tc.tile_pool(name="psum", bufs=2, space="PSUM"))
        pg = ctx.enter_context(tc.tile_pool(name="psum_g", bufs=1, space="PSUM"))
        # psum budget (8 banks): pp tags "mm_ps"(2) + "att"(2) + "tr"(2) + pg "gT"(1)

        def loadc(pool, src, shape, d, tag):
            t = pool.tile(shape, d, tag=tag)
            nc.sync.dma_start(t[:], src)
            return t

        id4_t = loadc(cp, id4, [4, 4], dt.float16, "id4")
        id32f_t = loadc(cp, id32f, [32, 32], dt.float32, "id32f")
        id128_t = loadc(cp, id128, [128, 128], dt.float16, "id128")
        ones_t = loadc(cp, ones_row, [1, SP], dt.float16, "ones")
        benc_row_t = loadc(cp, benc_row, [1, H], dt.float16, "bencr")
        benc_chk_t = loadc(cp, benc_chk, [1, 4, 128], dt.float16, "bencc")
        bcat_row_t = loadc(cp, bcat_row, [1, H], dt.float16, "bcatr")
        maskneg_t = loadc(cp, maskneg, [NB, SP], dt.float32, "maskneg")
        id4f_t = loadc(cp, id4f, [1, 4, 4], dt.float16, "id4f")
        id1_t = loadc(cp, id1, [1, 1], dt.float32, "id1")
        id4f32_t = loadc(cp, id4f32, [4, 4], dt.float32, "id4f32")

        w_hc_t = loadc(rp, w_hc, [128, 8, 4 * H], dt.float16, "whc")
        g1_t = loadc(rp, g1, [128, 16, NB, T], dt.float16, "g1")
        enc_t = loadc(rp, enc_tT, [128, 4, NB, SP], dt.float16, "enc")
        w_enc_t = loadc(rp, w_enc, [128, 4, H], dt.float16, "wenc")
        w_cat_t = loadc(rp, w_cat, [128, 8, H], dt.float16, "wcat")

        # state
        hist = rp.tile([128, 8, NB, T], dt.float16, tag="hist")
        hist_init = rp.tile([128, 8, NB], dt.float16, tag="hist_init")
        cst = rp.tile([128, 4, NB], dt.float32, tag="cst")
        energiesT = rp.tile([128, 4, NB, T], dt.float32r, tag="energiesT")
        mem = [rp.tile([128, 4, H], dt.float16, tag=f"mem{b}", name=f"mem{b}")
               for b in range(NB)]
        memT = [rp.tile([128, 4, SP], dt.float16, tag=f"memT{b}", name=f"memT{b}")
                for b in range(NB)]

        nc.sync.dma_start(hist_init[:], hist0)
        nc.sync.dma_start(cst[:], c0_2)

        # ---------------- phase 0: memories ----------------
        for b in range(NB):
            # memT[b][h, s] : out[m=h-chunk, n=s]
            for mc in range(4):
                pt = pp.tile([128, SP], dt.float32, tag="att")
                for kc in range(4):
                    nc.tensor.matmul(
                        pt[:], lhsT=w_enc_t[:, kc, mc * 128:(mc + 1) * 128],
                        rhs=enc_t[:, kc, b, :],
                        start=(kc == 0), stop=False)
                nc.tensor.matmul(pt[:], lhsT=benc_chk_t[:1, mc, :],
                                 rhs=ones_t[:1, :], start=False, stop=True)
                nc.vector.tensor_copy(memT[b][:, mc, :], pt[:])
            # mem[b][s, h] : out[m=s-chunk, n=h]
            for sc in range(4):
                pt = pp.tile([128, H], dt.float32, tag="att")
                for kc in range(4):
                    nc.tensor.matmul(
                        pt[:], lhsT=enc_t[:, kc, b, sc * 128:(sc + 1) * 128],
                        rhs=w_enc_t[:, kc, :],
                        start=(kc == 0), stop=False)
                nc.tensor.matmul(pt[:], lhsT=ones_t[:1, :128],
                                 rhs=benc_row_t[:1, :], start=False, stop=True)
                nc.vector.tensor_copy(mem[b][:, sc, :], pt[:])

        # ---------------- recurrence ----------------
        for t in range(T):
            # gates: per 512-chunk psum = G1[t] + [h|ctx]_t @ W_hc
            gev = wk.tile([NB, 4 * H], dt.float16, tag="gev")
            for n in range(4):
                nsl = slice(n * 512, (n + 1) * 512)
                gp = pp.tile([NB, 512], dt.float32, tag="mm_ps")
                hcur = hist_init if t == 0 else None
                for kc in range(8):
                    lhsT = (hist_init[:, kc, :] if t == 0
                            else hist[:, kc, :, t - 1])
                    nc.tensor.matmul(
                        gp[:], lhsT=lhsT,
                        rhs=w_hc_t[:, kc, nsl],
                        start=(kc == 0), stop=(kc == 7))
                nc.vector.tensor_copy(gev[:, nsl], gp[:])
            # transpose gates -> [128, 16ci, NB]
            gT = pg.tile([128, 16, NB], dt.float16, tag="gT")
            for ci in range(16):
                nc.tensor.transpose(gT[:, ci, :],
                                    gev[:, ci * 128:(ci + 1) * 128], id4_t[:])
            # add G1^T, then LSTM. sigma(x) = 0.5*tanh(x/2) + 0.5
            gs = wk.tile([128, 16, NB], dt.float32, tag="gs")
            nc.vector.tensor_tensor(gs[:], gT[:], g1_t[:, :, :, t], op=OP.add)
            tf_ = wk.tile([128, 4, NB], dt.float32, tag="tf")
            ti_ = wk.tile([128, 4, NB], dt.float32, tag="ti")
            to_ = wk.tile([128, 4, NB], dt.float32, tag="to")
            tg_ = wk.tile([128, 4, NB], dt.float32, tag="tg")
            nc.scalar.activation(ti_[:], gs[:, 0:4, :], AF.Tanh, scale=0.5)
            nc.scalar.activation(tf_[:], gs[:, 4:8, :], AF.Tanh, scale=0.5)
            nc.scalar.activation(tg_[:], gs[:, 8:12, :], AF.Tanh)
            nc.scalar.activation(to_[:], gs[:, 12:16, :], AF.Tanh, scale=0.5)
            u1 = wk.tile([128, 4, NB], dt.float32, tag="u1")
            u2 = wk.tile([128, 4, NB], dt.float32, tag="u2")
            # u1 = (tf+1) * v_prev  (= 4*sig(f)*c) ; u2 = (ti+1)*tg (= 2*sig(i)*tg)
            nc.vector.scalar_tensor_tensor(u1[:], tf_[:], 1.0, cst[:],
                                           op0=OP.add, op1=OP.mult)
            nc.vector.scalar_tensor_tensor(u2[:], ti_[:], 1.0, tg_[:],
                                           op0=OP.add, op1=OP.mult)
            # v = 2*c' = 0.5*u1 + u2
            nc.vector.scalar_tensor_tensor(cst[:], u1[:], 0.5, u2[:],
                                           op0=OP.mult, op1=OP.add)
            tc_ = wk.tile([128, 4, NB], dt.float32, tag="tc")
            nc.scalar.activation(tc_[:], cst[:], AF.Tanh, scale=0.5)
            h2 = wk.tile([128, 4, NB], dt.float32, tag="h2")
            nc.vector.scalar_tensor_tensor(h2[:], to_[:], 1.0, tc_[:],
                                           op0=OP.add, op1=OP.mult)
            nc.vector.tensor_scalar_mul(hist[:, 0:4, :, t], h2[:], 0.5)

            # energy: per-b M=1 matmul -> psum row 0 -> fp16 row -> stack
            est = pp.tile([NB, SP], dt.float32, tag="stk", bufs=1)
            for b in range(NB):
                ep = pp.tile([1, SP], dt.float32, tag="att")
                for kc in range(4):
                    nc.tensor.matmul(ep[:], lhsT=hist[:, kc, b:b + 1, t],
                                     rhs=memT[b][:, kc, :],
                                     start=(kc == 0), stop=(kc == 3))
                es_b = wk.tile([1, SP], dt.float16, tag="es", name="es_b")
                nc.vector.tensor_copy(es_b[:], ep[:])
                nc.tensor.matmul(est[:], lhsT=id4f_t[:1, b, :], rhs=es_b[:],
                                 start=(b == 0), stop=(b == NB - 1))
            # mask-add -> ec; archive (strided write); softmax
            ec = wk.tile([NB, SP], dt.float32, tag="ec")
            nc.vector.tensor_tensor(ec[:], est[:], maskneg_t[:], op=OP.add)
            etp = pp.tile([128, 4, NB], dt.float32, tag="tr")
            for sc in range(4):
                nc.tensor.transpose(etp[:, sc, :],
                                    ec[:, sc * 128:(sc + 1) * 128], id4f32_t[:])
            nc.vector.tensor_copy(energiesT[:, :, :, t], etp[:])
            nmx = wk.tile([NB, 1], dt.float32, tag="nmx")
            nc.vector.tensor_reduce(nmx[:], ec[:], axis=mybir.AxisListType.X,
                                    op=OP.max, negate=True)
            wexp = wk.tile([NB, SP], dt.float32, tag="wexp")
            den = wk.tile([NB, 1], dt.float32, tag="den")
            nc.scalar.activation(wexp[:], ec[:], AF.Exp, bias=nmx[:],
                                 accum_out=den[:])
            rec = wk.tile([NB, 1], dt.float32, tag="rec")
            nc.vector.reciprocal(rec[:], den[:])
            attn = wk.tile([NB, SP], dt.float16, tag="attn")
            nc.vector.tensor_scalar_mul(attn[:], wexp[:], rec[:])
            # wT [128, 4sc, NB]
            wTp = pp.tile([128, 4, NB], dt.float16, tag="tr")
            for sc in range(4):
                nc.tensor.transpose(wTp[:, sc, :],
                                    attn[:, sc * 128:(sc + 1) * 128], id4_t[:])
            wT = wk.tile([128, 4, NB], dt.float16, tag="wT")
            nc.vector.tensor_copy(wT[:], wTp[:])
            # ctx: per-b M=1 matmul -> fp16 row -> [1,128] transposes
            cTp = pp.tile([128, 4, NB], dt.float32, tag="tr")
            for b in range(NB):
                cpp = pp.tile([1, H], dt.float32, tag="att")
                for sc in range(4):
                    nc.tensor.matmul(cpp[:], lhsT=wT[:, sc, b:b + 1],
                                     rhs=mem[b][:, sc, :],
                                     start=(sc == 0), stop=(sc == 3))
                cs_b = wk.tile([1, H], dt.float32, tag="cs", name="cs_b")
                nc.vector.tensor_copy(cs_b[:], cpp[:])
                for hc in range(4):
                    nc.tensor.transpose(cTp[:, hc, b:b + 1],
                                        cs_b[:, hc * 128:(hc + 1) * 128],
                                        id1_t[:])
            nc.vector.tensor_copy(hist[:, 4:8, :, t], cTp[:])

        # ---------------- phase B ----------------
        # li = tanh([h|ctx] @ W_cat + b_cat)  [BT, H]
        lp = pp.tile([BT, H], dt.float32, tag="att")
        for kc in range(8):
            nc.tensor.matmul(lp[:], lhsT=hist[:, kc, :, :],
                             rhs=w_cat_t[:, kc, :],
                             start=(kc == 0), stop=False)
        nc.tensor.matmul(lp[:], lhsT=ones_t[:1, :BT], rhs=bcat_row_t[:1, :],
                         start=False, stop=True)
        li = wk.tile([BT, H], dt.float16, tag="li")
        nc.scalar.activation(li[:], lp[:], AF.Tanh)
        liTp = pp.tile([128, 4, BT], dt.float16, tag="tr")
        for hc in range(4):
            nc.tensor.transpose(liTp[:, hc, :],
                                li[:, hc * 128:(hc + 1) * 128], id128_t[:])
        liT = rp.tile([128, 4, BT], dt.float16, tag="liT")
        nc.vector.tensor_copy(liT[:], liTp[:])

        # dense logits, transposed out: psum [128v, BT]
        scr_r = out_scr.rearrange("b (m p) t -> m p b t", p=128)  # [NVC,128,NB,T]
        for m in range(NVC):
            wo = wk.tile([128, 4, 128], dt.float16, tag="wo", bufs=3)
            nc.sync.dma_start(wo[:], w_out_t[m])
            vp = pp.tile([128, BT], dt.float32, tag="mm_ps")
            for kc in range(4):
                nc.tensor.matmul(vp[:], lhsT=wo[:, kc, :], rhs=liT[:, kc, :],
                                 start=(kc == 0), stop=(kc == 3))
            mk = wk.tile([128, BT], dt.float32, tag="mk", bufs=3)
            nc.vector.tensor_scalar(mk[:], vp[:], 0.0, None, op0=OP.is_equal)
            ot = wk.tile([128, BT], dt.float32, tag="ot", bufs=3)
            nc.vector.scalar_tensor_tensor(ot[:], mk[:], float(NEG), vp[:],
                                           op0=OP.mult, op1=OP.add)
            nc.sync.dma_start(
                scr_r[m], ot[:].rearrange("p (b t) -> p b t", b=NB))

        # scatter deltas per b
        dT = wk.tile([128, NB, 4, T], dt.float32, tag="dT", bufs=1)
        for b in range(NB):
            wg_b = wk.tile([128, 4, U], dt.float16, tag="wgb")
            nc.sync.dma_start(wg_b[:], wg[b])
            a_b = wk.tile([128, 4, U], dt.float32r, tag="ab")
            nc.sync.dma_start(a_b[:], a_oh[b])
            # E2 = li_b @ Wg_b + energies^T A  (full final values at U_b)
            Ep = pp.tile([T, U], dt.float32, tag="att")
            for kc in range(4):
                nc.tensor.matmul(Ep[:], lhsT=liT[:, kc, b * T:(b + 1) * T],
                                 rhs=wg_b[:, kc, :],
                                 start=(kc == 0), stop=False)
            for sc in range(4):
                nc.tensor.matmul(Ep[:], lhsT=energiesT[:, sc, b, :],
                                 rhs=a_b[:, sc, :],
                                 start=False, stop=(sc == 3))
            m2 = wk.tile([T, U], dt.float32, tag="m2", bufs=1)
            nc.vector.tensor_scalar(m2[:], Ep[:], 0.0, None, op0=OP.is_equal)
            e2f = wk.tile([T, U], dt.float32, tag="e2f", bufs=1)
            nc.vector.scalar_tensor_tensor(e2f[:], m2[:], float(NEG), Ep[:],
                                           op0=OP.mult, op1=OP.add)
            dTp = pp.tile([128, 4, T], dt.float32, tag="tr")
            for uc in range(4):
                nc.tensor.transpose(dTp[:, uc, :],
                                    e2f[:, uc * 128:(uc + 1) * 128], id32f_t[:])
            nc.vector.tensor_copy(dT[:, b, :, :], dTp[:])

        nc.sync.dma_start(e2f_out.rearrange("p b u t -> p (b u t)"),
                          dT[:].rearrange("p b u t -> p (b u t)"))

    nc.compile()
    return nc


def _prep_core(inputs, core, folded):
    """Build the in_map for one core (its NB batches)."""
    (W_hc16, G1all, W_enc16, W_out_tiles, W_cat16, W_outP16) = folded
    bs = slice(core * NB, (core + 1) * NB)
    enc = _f32(inputs["encoder_outputs"][bs])          # [NB, S, H]
    encT = np.zeros((NB, H, SP), np.float16)
    encT[:, :, :S] = np.transpose(enc, (0, 2, 1)).astype(np.float16)
    enc_tT = np.ascontiguousarray(
        encT.reshape(NB, 4, 128, SP).transpose(2, 1, 0, 3))

    h0 = _f32(inputs["h0"][bs])                        # [NB, H]
    h0T = np.ascontiguousarray(h0.T.reshape(4, 128, NB).transpose(1, 0, 2))
    hist0 = np.zeros((128, 8, NB), np.float16)
    hist0[:, 0:4, :] = h0T.astype(np.float16)
    c0 = _f32(inputs["c0"][bs])
    c0_2 = np.ascontiguousarray(
        (2.0 * c0).T.reshape(4, 128, NB).transpose(1, 0, 2)).astype(np.float32)

    mask = np.asarray(inputs["encoder_mask"][bs])      # [NB, S] bool
    mn = np.zeros((NB, SP), np.float32)
    mn[:, :S][mask] = NEG
    mn[:, S:] = NEG

    g1c = _f16(G1all[bs].reshape(NB, T, 16, 128).transpose(3, 2, 0, 1))

    idx = np.asarray(inputs["ext_src_seq"][bs])        # [NB, S] int32
    wg = np.zeros((NB, 128, 4, U), np.float16)
    a_oh = np.zeros((NB, 128, 4, U), np.float32)
    for b in range(NB):
        ub = np.unique(idx[b])                         # sorted unique, <= 400
        nu = len(ub)
        pos = np.full(U, VE, np.int64)                 # pad -> discarded col
        pos[:nu] = ub
        # A one-hot [S->SP rows, U]: a[s, u] = 1 if idx[b,s] == pos[u]
        a = np.zeros((SP, U), np.float32)
        col = np.searchsorted(ub, idx[b])
        a[np.arange(S), col] = 1.0
        a_oh[b] = a.reshape(4, 128, U).transpose(1, 0, 2)
        wgb = W_outP16[:, pos]                         # [512, U] fp16
        wg[b] = wgb.reshape(4, 128, U).transpose(1, 0, 2)

    return {
        "enc_tT": enc_tT, "w_enc": W_enc16,
        "benc_row": _f16(inputs["b_enc"])[None, :],
        "benc_chk": _f16(inputs["b_enc"]).reshape(4, 128)[None],
        "ones_row": np.ones((1, SP), np.float16),
        "w_hc": W_hc16, "g1": g1c, "hist0": hist0, "c0_2": c0_2,
        "maskneg": mn, "w_cat": W_cat16,
        "bcat_row": _f16(inputs["b_cat"])[None, :],
        "w_out_t": W_out_tiles, "wg": wg, "a_oh": _tf32(a_oh),
        "id4": np.eye(4, dtype=np.float16),
        "id4f": np.eye(4, dtype=np.float16)[None],
        "id1": np.ones((1, 1), np.float32),
        "id4f32": np.eye(4, dtype=np.float32),
        "id32f": np.eye(32, dtype=np.float32),
        "id128": np.eye(128, dtype=np.float16),
    }


_LAST_FOLDED = None


def _fold(inputs):
    """Host-side weight folding shared across cores."""
    global _LAST_FOLDED
    W_red = _f32(inputs["W_red"])
    W_ih = _f32(inputs["W_ih"])
    W_ex = W_red[:E] @ W_ih                            # [300, 2048]
    W_cx = W_red[E:] @ W_ih                            # [512, 2048]
    b_all = (_f32(inputs["b_red"]) @ W_ih + _f32(inputs["b_ih"])
             + _f32(inputs["b_hh"]))
    W_hc = np.concatenate([_f32(inputs["W_hh"]), W_cx], 0)   # [1024, 2048]
    W_hc16 = _f16(W_hc.reshape(8, 128, 4 * H).transpose(1, 0, 2))

    emb_seq = _f32(inputs["embedding"])[np.asarray(inputs["trg_seq"])]  # [B,T,E]
    G1all = emb_seq.reshape(B * T, E) @ W_ex + b_all
    G1all = G1all.reshape(B, T, 4 * H)

    W_enc16 = _f16(_f32(inputs["W_enc"]).reshape(4, 128, H).transpose(1, 0, 2))
    W_cat16 = _f16(_f32(inputs["W_cat"]).reshape(8, 128, H).transpose(1, 0, 2))

    W_outP16 = np.zeros((H, VP), np.float16)
    W_outP16[:, :V] = _f16(inputs["W_out"])
    W_out_tiles = np.ascontiguousarray(
        W_outP16.reshape(4, 128, NVC, 128).transpose(2, 1, 0, 3))
    b_out = _f32(inputs["b_out"])
    assert not np.any(b_out), "kernel build assumes b_out == 0"

    folded = (W_hc16, G1all, W_enc16, W_out_tiles, W_cat16, W_outP16)
    _LAST_FOLDED = folded
    return folded


def kernel(**inputs):
    global _BUILT
    from concourse.bass_utils import run_bass_kernel_spmd

    folded = _fold(inputs)

    if _BUILT is None:
        _BUILT = _build()
    nc = _BUILT

    in_maps = [_prep_core(inputs, core, folded) for core in range(NC)]
    res = run_bass_kernel_spmd(nc, in_maps, list(range(NC)))

    idx_all = np.asarray(inputs["ext_src_seq"])
    out = np.empty((B, T, VE), np.float32)
    for core in range(NC):
        scr = np.array(res.results[core]["out_scr"])   # [NB, VP, T]
        e2f = res.results[core]["e2f_out"]             # [128, NB, 4, T]
        for b in range(NB):
            gb = core * NB + b
            ub = np.unique(idx_all[gb])
            pos = np.full(U, VE, np.int64)
            pos[:len(ub)] = ub
            scr[b, pos, :] = e2f[:, b, :, :].transpose(1, 0, 2).reshape(U, T)
        out[core * NB:(core + 1) * NB] = np.transpose(
            scr[:, :VE, :], (0, 2, 1))
    return out
